# revision 1
# baseline (speedup 1.0000x reference)
"""Bass/Trainium2 kernel for nn_KPlexPool (GCN blocks + cover pooling), 8 NeuronCores.

Strategy: shard nodes/clusters across 8 cores by slicing each graph's
(batch-sorted) range 8 ways, degree-sorted within slices for padded-CSR
efficiency. Per GCN layer: per-shard matmul h=x@W (TensorE), pre-scale rows by
dis=rsqrt(deg) -> AllGather hs -> padded-CSR indirect-DMA gather of source rows
-> DVE multiply by edge weight + segmented reduce -> scale/bias/relu epilogue.
Cover pooling (sum+max) and batch readouts reuse the same CSR machinery.
Tiny AllReduces combine the [16,256] readout; every core computes the final
BN+MLP+softmax head identically.  All index structure is host-prepared
(sharding); all FLOPs happen on device.
"""

import sys
import numpy as np

sys.path.insert(0, "/opt/trn_rl_repo")

PART = 128
NCORES = 8
EPS = 1e-5
S_COLS = 64          # max slot-columns per gather chunk (128*64 = 8192 slots)
G_MAX = 32           # max tiles per gather chunk


# ----------------------------------------------------------------- host prep

def _shard_items(batch, sortkey, B):
    """Slice each graph's contiguous (sorted-batch) range into 8 balanced
    parts; within a slice order items by sortkey desc; pad each graph-slice to
    a uniform (across cores) multiple of 128 rows.

    Returns perm [NCORES][rows] (orig ids, -1 pad), pos [n] -> global row,
    rows, tile_graph [T_tot]."""
    n = batch.shape[0]
    counts = np.bincount(batch, minlength=B)
    starts = np.concatenate([[0], np.cumsum(counts)[:-1]])
    perm_cores = [[] for _ in range(NCORES)]
    tile_graph = []
    for g in range(B):
        cnt = int(counts[g])
        st = int(starts[g])
        base, rem = divmod(cnt, NCORES)
        sizes = [base + (1 if c < rem else 0) for c in range(NCORES)]
        Tg = max(1, -(-max(sizes) // PART))
        tile_graph += [g] * Tg
        off = st
        for c in range(NCORES):
            s = sizes[c]
            ids = np.arange(off, off + s)
            off += s
            order = np.argsort(-sortkey[ids], kind="stable")
            padded = np.full(Tg * PART, -1, dtype=np.int64)
            padded[:s] = ids[order]
            perm_cores[c].append(padded)
    perm = [np.concatenate(p) for p in perm_cores]
    rows = perm[0].shape[0]
    pos = np.full(n, -1, dtype=np.int64)
    for c in range(NCORES):
        real = perm[c] >= 0
        pos[perm[c][real]] = c * rows + np.nonzero(real)[0]
    return perm, pos, rows, np.asarray(tile_graph)


def _csr_build(dst_local_all, src_pos_all, w_all, owner_all, rows, pad_idx):
    """Build per-core padded CSR. dst_local_all/src_pos_all/w_all: per-edge
    (global) arrays; owner_all: owning core per edge. Returns k_t (uniform),
    col_off, and per-core (idx [128,S], ew [128,S]) arrays."""
    T = rows // PART
    k_core = np.zeros((NCORES, T), dtype=np.int64)
    per_core = []
    for c in range(NCORES):
        m = owner_all == c
        dl = dst_local_all[m]
        sp = src_pos_all[m]
        w = w_all[m] if w_all is not None else None
        cnt = np.bincount(dl, minlength=rows)
        k_core[c] = cnt.reshape(T, PART).max(axis=1)
        per_core.append((dl, sp, w, cnt))
    k_t = np.maximum(k_core.max(axis=0), 1)
    col_off = np.concatenate([[0], np.cumsum(k_t)]).astype(np.int64)
    S = int(col_off[-1])
    idxs, ews = [], []
    for c in range(NCORES):
        dl, sp, w, cnt = per_core[c]
        order = np.argsort(dl, kind="stable")
        dls, sps = dl[order], sp[order]
        first = np.concatenate([[0], np.cumsum(cnt)[:-1]])
        rank = np.arange(dls.shape[0]) - first[dls]
        tile = dls // PART
        p = dls % PART
        col = col_off[tile] + rank
        idx = np.full((PART, S), pad_idx, dtype=np.int32)
        ew = np.zeros((PART, S), dtype=np.float32)
        idx[p, col] = sps.astype(np.int32)
        if w is not None:
            ew[p, col] = w[order]
        idxs.append(idx)
        ews.append(ew)
    return k_t, col_off, S, idxs, ews


def _chunks(k_t, col_off):
    """Greedy grouping of consecutive tiles into gather chunks."""
    out = []
    t0 = 0
    T = len(k_t)
    while t0 < T:
        t1 = t0
        while (t1 < T and col_off[t1 + 1] - col_off[t0] <= S_COLS
               and t1 - t0 < G_MAX):
            t1 += 1
        assert t1 > t0, f"tile {t0} k={k_t[t0]} exceeds S_COLS={S_COLS}"
        out.append((t0, t1, int(col_off[t0]), int(col_off[t1])))
        t0 = t1
    return out


def _prep(inputs):
    f32 = np.float32
    x = np.asarray(inputs["x"], f32)
    ei = np.asarray(inputs["edge_index"], np.int64)
    wts = np.asarray(inputs["weights"], f32)
    batch = np.asarray(inputs["batch"], np.int64)
    cover_n = np.asarray(inputs["cover_n"], np.int64)
    cover_c = np.asarray(inputs["cover_c"], np.int64)
    ei2 = np.asarray(inputs["edge_index2"], np.int64)
    wts2 = np.asarray(inputs["weights2"], f32)
    batch2 = np.asarray(inputs["batch2"], np.int64)
    N = x.shape[0]
    C = batch2.shape[0]
    B = int(batch.max()) + 1 if batch.size else 1
    B = max(B, int(batch2.max()) + 1)

    indeg = np.bincount(ei[1], minlength=N)
    perm1, pos1, rows1, tg1 = _shard_items(batch, indeg, B)
    key2 = np.bincount(ei2[1], minlength=C) + np.bincount(cover_c, minlength=C)
    perm2, pos2, rows2, tg2 = _shard_items(batch2, key2, B)
    T1, T2 = rows1 // PART, rows2 // PART

    # a guaranteed pad (zero) row in the node arrangement
    zero_pos = None
    for c in range(NCORES):
        pads = np.nonzero(perm1[c] < 0)[0]
        if pads.size:
            zero_pos = c * rows1 + int(pads[0])
            break
    assert zero_pos is not None

    # big-graph CSR (dest = col = ei[1]); owner by dest position
    dpos = pos1[ei[1]]
    owner = dpos // rows1
    dloc = dpos % rows1
    k1, coff1, S1, idx1, ew1 = _csr_build(dloc, pos1[ei[0]], wts, owner, rows1, 0)

    # cover CSR (dest = cluster), sources are node positions into x1_full
    cpos = pos2[cover_c]
    ownc = cpos // rows2
    clol = cpos % rows2
    kc, coffc, Sc, idxc, _ = _csr_build(clol, pos1[cover_n], None, ownc, rows2,
                                        zero_pos)

    # pooled-graph CSR
    dpos2 = pos2[ei2[1]]
    own2 = dpos2 // rows2
    dlo2 = dpos2 % rows2
    k2, coff2, S2, idx2, ew2 = _csr_build(dlo2, pos2[ei2[0]], wts2, own2, rows2, 0)

    # per-core dense inputs
    xs, m1s, m2s = [], [], []
    for c in range(NCORES):
        pc = perm1[c]
        xc = np.zeros((rows1, x.shape[1]), f32)
        xc[pc >= 0] = x[pc[pc >= 0]]
        xs.append(xc)
        m1s.append(np.ascontiguousarray(
            (pc >= 0).astype(f32).reshape(T1, PART).T))
        p2 = perm2[c]
        m2s.append(np.ascontiguousarray(
            (p2 >= 0).astype(f32).reshape(T2, PART).T))

    meta = dict(B=B, T1=T1, T2=T2, rows1=rows1, rows2=rows2,
                k1=k1, coff1=coff1, S1=S1, ch1=_chunks(k1, coff1),
                kc=kc, coffc=coffc, Sc=Sc, chc=_chunks(kc, coffc),
                k2=k2, coff2=coff2, S2=S2, ch2=_chunks(k2, coff2),
                tg1=tg1, tg2=tg2, FIN=x.shape[1])

    # weights, replicated biases, permuted head params
    rep = lambda v: np.ascontiguousarray(
        np.broadcast_to(np.asarray(v, f32).reshape(1, -1), (PART, v.shape[-1])))
    g = np.asarray(inputs["bn_gamma"], f32)
    bb = np.asarray(inputs["bn_beta"], f32)
    l1w = np.asarray(inputs["lin1_W"], f32)
    H = np.asarray(inputs["W_in0"], f32).shape[1]
    selS = np.r_[0:H, 2 * H:3 * H]
    selM = np.r_[H:2 * H, 3 * H:4 * H]
    shared = {
        "W_in0": np.asarray(inputs["W_in0"], f32),
        "W_in1": np.asarray(inputs["W_in1"], f32),
        "Wl_in": np.asarray(inputs["Wl_in"], f32),
        "W_b0": np.asarray(inputs["W_b0"], f32),
        "W_b1": np.asarray(inputs["W_b1"], f32),
        "Wl_b": np.asarray(inputs["Wl_b"], f32),
        "b_in0": rep(inputs["b_in0"]), "b_in1": rep(inputs["b_in1"]),
        "bl_in": rep(inputs["bl_in"]), "b_b0": rep(inputs["b_b0"]),
        "b_b1": rep(inputs["b_b1"]), "bl_b": rep(inputs["bl_b"]),
        "gammaS": np.ascontiguousarray(g[selS].reshape(PART, 1)),
        "gammaM": np.ascontiguousarray(g[selM].reshape(PART, 1)),
        "betaS": np.ascontiguousarray(bb[selS].reshape(PART, 1)),
        "betaM": np.ascontiguousarray(bb[selM].reshape(PART, 1)),
        "l1WS": np.ascontiguousarray(l1w[selS]),
        "l1WM": np.ascontiguousarray(l1w[selM]),
        "l1b": rep(inputs["lin1_b"]),
        "l2W": np.asarray(inputs["lin2_W"], f32),
        "l2b": rep(inputs["lin2_b"]),
    }
    in_maps = []
    for c in range(NCORES):
        m = dict(shared)
        m["x_c"] = xs[c]
        m["mask1"] = m1s[c]
        m["mask2"] = m2s[c]
        m["idx1"] = idx1[c]
        m["ew1"] = ew1[c]
        m["idxc"] = idxc[c]
        m["idx2"] = idx2[c]
        m["ew2"] = ew2[c]
        in_maps.append(m)
    return meta, in_maps


# ------------------------------------------------------------- device kernel

def _build(meta, NCLS=10, H=64):
    import concourse.bass as bass
    import concourse.bacc as bacc
    import concourse.mybir as mybir
    import concourse.tile as tile
    from concourse.masks import make_identity

    f32 = mybir.dt.float32
    i32 = mybir.dt.int32
    ALU = mybir.AluOpType
    ACTF = mybir.ActivationFunctionType
    AX = mybir.AxisListType
    IOA = bass.IndirectOffsetOnAxis

    B = meta["B"]
    T1, T2 = meta["T1"], meta["T2"]
    rows1, rows2 = meta["rows1"], meta["rows2"]
    FIN = meta["FIN"]
    RG = [list(range(NCORES))]

    nc = bacc.Bacc("TRN2", target_bir_lowering=False, debug=False,
                   num_devices=NCORES, num_swdge_queues=4)

    # Round-robin indirect DMAs across the 4 SWDGE queues: descriptor
    # generation (the bottleneck at ~8.5ns/row on one queue) parallelizes
    # ~3x across queues.
    _qctr = [0]

    def _q(inst):
        i = _qctr[0] % 4
        _qctr[0] += 1
        if i:
            inst.ins.queue = f"qPoolDynamic{i}"
        return inst

    # ---- I/O ----
    ein = lambda n, s, d=f32: nc.dram_tensor(n, s, d, kind="ExternalInput")
    x_c = ein("x_c", [rows1, FIN])
    mask1 = ein("mask1", [PART, T1])
    mask2 = ein("mask2", [PART, T2])
    idx1 = ein("idx1", [PART, meta["S1"]], i32)
    ew1 = ein("ew1", [PART, meta["S1"]])
    idxc = ein("idxc", [PART, meta["Sc"]], i32)
    idx2 = ein("idx2", [PART, meta["S2"]], i32)
    ew2 = ein("ew2", [PART, meta["S2"]])
    wshapes = {"W_in0": [FIN, H], "W_in1": [H, H], "Wl_in": [2 * H, H],
               "W_b0": [2 * H, H], "W_b1": [H, H], "Wl_b": [2 * H, H]}
    Ws = {n: ein(n, s) for n, s in wshapes.items()}
    bs = {n: ein(n, [PART, H]) for n in
          ("b_in0", "b_in1", "bl_in", "b_b0", "b_b1", "bl_b")}
    gammaS = ein("gammaS", [PART, 1]); gammaM = ein("gammaM", [PART, 1])
    betaS = ein("betaS", [PART, 1]); betaM = ein("betaM", [PART, 1])
    l1WS = ein("l1WS", [PART, H]); l1WM = ein("l1WM", [PART, H])
    l1b = ein("l1b", [PART, H])
    l2W = ein("l2W", [H, NCLS]); l2b = ein("l2b", [PART, NCLS])
    out_ext = nc.dram_tensor("out", [B, NCLS], f32, kind="ExternalOutput")

    # ---- internal DRAM ----
    hs_c1 = nc.dram_tensor("hs_c1", [rows1, H], f32)
    hs_full1 = nc.dram_tensor("hs_full1", [NCORES * rows1, H], f32, addr_space="Shared")
    hs_c1b = nc.dram_tensor("hs_c1b", [rows1, H], f32)
    hs_full1b = nc.dram_tensor("hs_full1b", [NCORES * rows1, H], f32, addr_space="Shared")
    x1_c = nc.dram_tensor("x1_c", [rows1, H], f32)
    x1_full = nc.dram_tensor("x1_full", [NCORES * rows1, H], f32, addr_space="Shared")
    hs_c2 = nc.dram_tensor("hs_c2", [rows2, H], f32)
    hs_full2 = nc.dram_tensor("hs_full2", [NCORES * rows2, H], f32, addr_space="Shared")
    hs_c2b = nc.dram_tensor("hs_c2b", [rows2, H], f32)
    hs_full2b = nc.dram_tensor("hs_full2b", [NCORES * rows2, H], f32, addr_space="Shared")
    g_dram1 = nc.dram_tensor("g_dram1", [rows1, 2 * H], f32)
    g_dram2 = nc.dram_tensor("g_dram2", [rows2, 2 * H], f32)
    arS_in = nc.dram_tensor("arS_in", [PART, B], f32)
    arS_out = nc.dram_tensor("arS_out", [PART, B], f32, addr_space="Shared")
    arM_in = nc.dram_tensor("arM_in", [PART, B], f32)
    arM_out = nc.dram_tensor("arM_out", [PART, B], f32, addr_space="Shared")

    with tile.TileContext(nc) as tc:
        with (tc.tile_pool(name="const", bufs=1) as cpool,
              tc.tile_pool(name="res", bufs=1) as rpool,
              tc.tile_pool(name="work", bufs=3) as wpool,
              tc.tile_pool(name="ps", bufs=2, space="PSUM") as pspool,
              tc.tile_pool(name="psacc", bufs=1, space="PSUM") as papool):

            ident = cpool.tile([PART, PART], f32, tag="ident")
            make_identity(nc, ident[:])

            # resident SBUF
            def load2d(dram, shape, dt=f32, tag=None):
                t = cpool.tile(list(shape), dt, tag=tag or dram.name)
                nc.sync.dma_start(t[:], dram[:, :])
                return t

            idx1_s = load2d(idx1, (PART, meta["S1"]), i32)
            ew1_s = load2d(ew1, (PART, meta["S1"]))
            idxc_s = load2d(idxc, (PART, meta["Sc"]), i32)
            idx2_s = load2d(idx2, (PART, meta["S2"]), i32)
            ew2_s = load2d(ew2, (PART, meta["S2"]))
            mask1_s = load2d(mask1, (PART, T1))
            mask2_s = load2d(mask2, (PART, T2))
            W_s = {n: load2d(Ws[n], Ws[n].shape) for n in Ws}
            b_s = {n: load2d(bs[n], (PART, H)) for n in bs}
            l1WS_s = load2d(l1WS, (PART, H)); l1WM_s = load2d(l1WM, (PART, H))
            l1b_s = load2d(l1b, (PART, H))
            l2W_s = load2d(l2W, (H, NCLS)); l2b_s = load2d(l2b, (PART, NCLS))
            gS_s = load2d(gammaS, (PART, 1)); gM_s = load2d(gammaM, (PART, 1))
            bS_s = load2d(betaS, (PART, 1)); bM_s = load2d(betaM, (PART, 1))

            dis1 = rpool.tile([PART, T1], f32, tag="dis1")
            xp = rpool.tile([PART, T2, 2, H], f32, tag="xp")
            dis2 = rpool.tile([PART, T2], f32, tag="dis2")
            rm1 = rpool.tile([PART, B, H], f32, tag="rm1")
            rm2 = rpool.tile([PART, B, H], f32, tag="rm2")
            oneh = rpool.tile([PART, B, B], f32, tag="oneh")
            nc.vector.memset(rm1[:], 0.0)
            nc.vector.memset(rm2[:], 0.0)
            nc.vector.memset(oneh[:], 0.0)
            for g in range(B):
                nc.vector.memset(oneh[:, g, g:g + 1], 1.0)

            def bc_mid(ap2d, G):
                """[128, F] AP -> [128, (0-step G), F] broadcast view."""
                a = ap2d.ap
                return bass.AP(ap2d.tensor, ap2d.offset,
                               [a[0], [0, G], a[-1]])

            ps_sum1 = papool.tile([B, H], f32, tag="sum1")
            ps_sum2 = papool.tile([B, H], f32, tag="sum2")

            # degree -> dis (= rsqrt(deg) * mask)
            def make_dis(ew_s, k, coff, T, mask_s, dis_t):
                deg = wpool.tile([PART, max(T1, T2)], f32, tag="deg")
                for t in range(T):
                    kk = int(k[t])
                    nc.vector.tensor_reduce(
                        out=deg[:, t:t + 1],
                        in_=ew_s[:, int(coff[t]):int(coff[t]) + kk],
                        axis=AX.X, op=ALU.add)
                nc.vector.tensor_scalar_add(deg[:, :T], deg[:, :T], 1.0)
                nc.vector.reciprocal(deg[:, :T], deg[:, :T])
                nc.scalar.activation(deg[:, :T], deg[:, :T], ACTF.Sqrt)
                nc.vector.tensor_tensor(out=dis_t[:], in0=deg[:, :T],
                                        in1=mask_s[:], op=ALU.mult)

            make_dis(ew1_s, meta["k1"], meta["coff1"], T1, mask1_s, dis1)
            make_dis(ew2_s, meta["k2"], meta["coff2"], T2, mask2_s, dis2)

            xv = x_c.ap().rearrange("(t p) f -> p t f", p=PART)
            gd1 = g_dram1.ap().rearrange("(t p) f -> p t f", p=PART)
            gd2 = g_dram2.ap().rearrange("(t p) f -> p t f", p=PART)

            # h = act @ W, hs = dis * h -> DRAM hs_c.  act_src: DRAM AP
            # viewed [p t f] (pass None + act_sb for an SBUF source).
            def matmul_scale(act_src, Tn, Fin, W, dis_t, hs_dram, act_sb=None):
                hsd = hs_dram.ap().rearrange("(t p) f -> p t f", p=PART)
                for t in range(Tn):
                    if act_sb is None:
                        at = wpool.tile([PART, PART], f32, tag="at")
                        nc.sync.dma_start(at[:, :Fin], act_src[:, t, :])
                        av = at[:, :Fin]
                    else:
                        av = act_sb[:, t, :]
                    tp = pspool.tile([PART, PART], f32, tag="tp")
                    nc.tensor.transpose(tp[:Fin, :], av, ident[:])
                    tsb = wpool.tile([PART, PART], f32, tag="tsb")
                    nc.scalar.copy(out=tsb[:Fin, :], in_=tp[:Fin, :])
                    mm = pspool.tile([PART, H], f32, tag="mm")
                    nc.tensor.matmul(out=mm[:], lhsT=tsb[:Fin, :], rhs=W[:],
                                     start=True, stop=True)
                    hst = wpool.tile([PART, 1, H], f32, tag="hst")
                    nc.vector.tensor_scalar(
                        out=hst[:, 0, :], in0=mm[:],
                        scalar1=dis_t[:, t:t + 1], scalar2=None, op0=ALU.mult)
                    nc.sync.dma_start(hsd[:, t:t + 1, :], hst[:])

            def allgather(src, dst):
                nc.gpsimd.collective_compute(
                    "AllGather", ALU.bypass, ins=[src.ap().opt()],
                    outs=[dst.ap().opt()], replica_groups=RG)

            # gather + ew-mult + per-tile reduce + epilogue -> gout
            def message_pass(hs_full, idx_s, ew_s, k, chunks, coff, dis_t,
                             hs_dram, bias, gout_dram):
                hsd = hs_dram.ap().rearrange("(t p) f -> p t f", p=PART)
                for (t0, t1, c0, c1) in chunks:
                    Wc = c1 - c0
                    G = t1 - t0
                    gt = wpool.tile([PART, S_COLS, H], f32, tag="gt")
                    gv = gt[:, :Wc, :]
                    for cc in range(Wc):
                        _q(nc.gpsimd.indirect_dma_start(
                            out=gt[:, cc, :], out_offset=None,
                            in_=hs_full[:, :],
                            in_offset=IOA(ap=idx_s[:, c0 + cc:c0 + cc + 1],
                                          axis=0)))
                    if ew_s is not None:
                        nc.vector.tensor_tensor(
                            out=gv, in0=gv,
                            in1=ew_s[:, c0:c1].to_broadcast([PART, Wc, H]),
                            op=ALU.mult)
                    rt = wpool.tile([PART, G_MAX, H], f32, tag="rt")
                    for j, t in enumerate(range(t0, t1)):
                        ca = int(coff[t]) - c0
                        kk = int(k[t])
                        nc.vector.tensor_reduce(
                            out=rt[:, j, :],
                            in_=gt[:, ca:ca + kk, :].rearrange(
                                "p k f -> p f k"),
                            axis=AX.X, op=ALU.add)
                    zv = rt[:, :G, :]
                    hsc = wpool.tile([PART, G_MAX, H], f32, tag="hsc")
                    nc.sync.dma_start(hsc[:, :G, :], hsd[:, t0:t1, :])
                    nc.vector.tensor_tensor(out=zv, in0=zv,
                                            in1=hsc[:, :G, :], op=ALU.add)
                    nc.vector.tensor_tensor(
                        out=zv, in0=zv,
                        in1=dis_t[:, t0:t1].to_broadcast([PART, G, H]),
                        op=ALU.mult)
                    nc.vector.tensor_tensor(out=zv, in0=zv,
                                            in1=bc_mid(bias[:], G), op=ALU.add)
                    go = wpool.tile([PART, G_MAX, H], f32, tag="go")
                    nc.scalar.activation(go[:, :G, :], zv, ACTF.Relu)
                    nc.sync.dma_start(gout_dram[:, t0:t1, :], go[:, :G, :])

            # ---------- big block ----------
            with nc.named_scope("mm1a"):
                matmul_scale(xv, T1, FIN, W_s["W_in0"][:], dis1, hs_c1)
            with nc.named_scope("ag1a"):
                allgather(hs_c1, hs_full1)
            with nc.named_scope("mp1a"):
                message_pass(hs_full1, idx1_s, ew1_s, meta["k1"], meta["ch1"],
                             meta["coff1"], dis1, hs_c1, b_s["b_in0"],
                             gd1[:, :, 0:H])
            with nc.named_scope("mm1b"):
                matmul_scale(gd1[:, :, 0:H], T1, H, W_s["W_in1"][:], dis1,
                             hs_c1b)
            with nc.named_scope("ag1b"):
                allgather(hs_c1b, hs_full1b)
            with nc.named_scope("mp1b"):
                message_pass(hs_full1b, idx1_s, ew1_s, meta["k1"], meta["ch1"],
                             meta["coff1"], dis1, hs_c1b, b_s["b_in1"],
                             gd1[:, :, H:2 * H])

            # JK cat + linear + relu*mask + readout + DMA x1
            tg1 = meta["tg1"]
            x1v = x1_c.ap().rearrange("(t p) f -> p t f", p=PART)
            sc_jk1 = nc.named_scope("jk1"); sc_jk1.__enter__()
            for t in range(T1):
                cat = wpool.tile([PART, 2 * H], f32, tag="at")
                nc.sync.dma_start(cat[:], g_dram1[t * PART:(t + 1) * PART, :])
                tp = pspool.tile([PART, PART], f32, tag="tp")
                nc.tensor.transpose(tp[:], cat[:], ident[:])
                tsb = wpool.tile([PART, PART], f32, tag="tsb")
                nc.scalar.copy(out=tsb[:], in_=tp[:])
                mm = pspool.tile([PART, 2 * H], f32, tag="mm")
                nc.tensor.matmul(out=mm[:, :H], lhsT=tsb[:],
                                 rhs=W_s["Wl_in"][:], start=True, stop=True)
                x1t = wpool.tile([PART, 1, H], f32, tag="hst")
                nc.vector.tensor_tensor(out=x1t[:, 0, :], in0=mm[:, :H],
                                        in1=b_s["bl_in"][:], op=ALU.add)
                nc.scalar.activation(x1t[:, 0, :], x1t[:, 0, :], ACTF.Relu,
                                     scale=mask1_s[:, t:t + 1])
                g = int(tg1[t])
                nc.tensor.matmul(out=ps_sum1[:], lhsT=oneh[:, g, :],
                                 rhs=x1t[:, 0, :], start=(t == 0),
                                 stop=(t == T1 - 1), skip_group_check=True)
                nc.vector.tensor_tensor(out=rm1[:, g, :], in0=rm1[:, g, :],
                                        in1=x1t[:, 0, :], op=ALU.max)
                nc.sync.dma_start(x1v[:, t:t + 1, :], x1t[:])
            sc_jk1.__exit__(None, None, None)
            with nc.named_scope("agx1"):
                allgather(x1_c, x1_full)

            # ---------- cover pooling ----------
            sc_cov = nc.named_scope("cover"); sc_cov.__enter__()
            for (t0, t1, c0, c1) in meta["chc"]:
                Wc = c1 - c0
                gt = wpool.tile([PART, S_COLS, H], f32, tag="gt")
                for cc in range(Wc):
                    _q(nc.gpsimd.indirect_dma_start(
                        out=gt[:, cc, :], out_offset=None, in_=x1_full[:, :],
                        in_offset=IOA(ap=idxc_s[:, c0 + cc:c0 + cc + 1],
                                      axis=0)))
                for j, t in enumerate(range(t0, t1)):
                    ca = int(meta["coffc"][t]) - c0
                    kk = int(meta["kc"][t])
                    view = gt[:, ca:ca + kk, :].rearrange("p k f -> p f k")
                    nc.vector.tensor_reduce(out=xp[:, t, 0, :], in_=view,
                                            axis=AX.X, op=ALU.add)
                    nc.vector.tensor_reduce(out=xp[:, t, 1, :], in_=view,
                                            axis=AX.X, op=ALU.max)

            sc_cov.__exit__(None, None, None)

            # ---------- pooled block ----------
            with nc.named_scope("mm2a"):
                matmul_scale(None, T2, 2 * H, W_s["W_b0"][:], dis2, hs_c2,
                             act_sb=xp[:].rearrange("p t a b -> p t (a b)"))
            with nc.named_scope("ag2a"):
                allgather(hs_c2, hs_full2)
            with nc.named_scope("mp2a"):
                message_pass(hs_full2, idx2_s, ew2_s, meta["k2"], meta["ch2"],
                             meta["coff2"], dis2, hs_c2, b_s["b_b0"],
                             gd2[:, :, 0:H])
            with nc.named_scope("mm2b"):
                matmul_scale(gd2[:, :, 0:H], T2, H, W_s["W_b1"][:], dis2,
                             hs_c2b)
            with nc.named_scope("ag2b"):
                allgather(hs_c2b, hs_full2b)
            with nc.named_scope("mp2b"):
                message_pass(hs_full2b, idx2_s, ew2_s, meta["k2"], meta["ch2"],
                             meta["coff2"], dis2, hs_c2b, b_s["b_b1"],
                             gd2[:, :, H:2 * H])

            tg2 = meta["tg2"]
            sc_jk2 = nc.named_scope("jk2"); sc_jk2.__enter__()
            for t in range(T2):
                cat = wpool.tile([PART, 2 * H], f32, tag="at")
                nc.sync.dma_start(cat[:], g_dram2[t * PART:(t + 1) * PART, :])
                tp = pspool.tile([PART, PART], f32, tag="tp")
                nc.tensor.transpose(tp[:], cat[:], ident[:])
                tsb = wpool.tile([PART, PART], f32, tag="tsb")
                nc.scalar.copy(out=tsb[:], in_=tp[:])
                mm = pspool.tile([PART, 2 * H], f32, tag="mm")
                nc.tensor.matmul(out=mm[:, :H], lhsT=tsb[:],
                                 rhs=W_s["Wl_b"][:], start=True, stop=True)
                x2t = wpool.tile([PART, 1, H], f32, tag="hst")
                nc.vector.tensor_tensor(out=x2t[:, 0, :], in0=mm[:, :H],
                                        in1=b_s["bl_b"][:], op=ALU.add)
                nc.scalar.activation(x2t[:, 0, :], x2t[:, 0, :], ACTF.Relu,
                                     scale=mask2_s[:, t:t + 1])
                g = int(tg2[t])
                nc.tensor.matmul(out=ps_sum2[:], lhsT=oneh[:, g, :],
                                 rhs=x2t[:, 0, :], start=(t == 0),
                                 stop=(t == T2 - 1), skip_group_check=True)
                nc.vector.tensor_tensor(out=rm2[:, g, :], in0=rm2[:, g, :],
                                        in1=x2t[:, 0, :], op=ALU.max)

            sc_jk2.__exit__(None, None, None)

            # ---------- readout combine + head ----------
            sc_head = nc.named_scope("head"); sc_head.__enter__()
            # sums: psacc "sum1" [B, H] / "sum2"; transpose -> [H, B]
            sum1_sb = wpool.tile([B, H], f32, tag="s1sb")
            nc.scalar.copy(out=sum1_sb[:], in_=ps_sum1[:])
            sum2_sb = wpool.tile([B, H], f32, tag="s2sb")
            nc.scalar.copy(out=sum2_sb[:], in_=ps_sum2[:])
            sT = pspool.tile([H, B], f32, tag="tp")
            nc.tensor.matmul(out=sT[:], lhsT=sum1_sb[:], rhs=ident[:B, :B],
                             start=True, stop=True)
            sT1 = wpool.tile([H, B], f32, tag="sT1")
            nc.scalar.copy(out=sT1[:], in_=sT[:])
            sT_2 = pspool.tile([H, B], f32, tag="tp")
            nc.tensor.matmul(out=sT_2[:], lhsT=sum2_sb[:], rhs=ident[:B, :B],
                             start=True, stop=True)
            sT2 = wpool.tile([H, B], f32, tag="sT2")
            nc.scalar.copy(out=sT2[:], in_=sT_2[:])
            nc.sync.dma_start(arS_in[0:H, :], sT1[:])
            nc.sync.dma_start(arS_in[H:2 * H, :], sT2[:])

            # maxes: rm[g] [128, H] -> transpose -> reduce-max -> [H, 1]
            mT1 = wpool.tile([H, B], f32, tag="mT1")
            mT2 = wpool.tile([H, B], f32, tag="mT2")
            for g in range(B):
                for rm, mt in ((rm1, mT1), (rm2, mT2)):
                    tpm = pspool.tile([H, PART], f32, tag="tp")
                    nc.tensor.transpose(tpm[:], rm[:, g, :], ident[:])
                    msb = wpool.tile([H, PART], f32, tag="msb")
                    nc.scalar.copy(out=msb[:], in_=tpm[:])
                    nc.vector.tensor_reduce(out=mt[:, g:g + 1], in_=msb[:],
                                            axis=AX.X, op=ALU.max)
            nc.sync.dma_start(arM_in[0:H, :], mT1[:])
            nc.sync.dma_start(arM_in[H:2 * H, :], mT2[:])

            nc.gpsimd.collective_compute(
                "AllReduce", ALU.add, ins=[arS_in.ap().opt()],
                outs=[arS_out.ap().opt()], replica_groups=RG)
            nc.gpsimd.collective_compute(
                "AllReduce", ALU.max, ins=[arM_in.ap().opt()],
                outs=[arM_out.ap().opt()], replica_groups=RG)

            S_sb = wpool.tile([PART, B], f32, tag="Ssb")
            M_sb = wpool.tile([PART, B], f32, tag="Msb")
            nc.sync.dma_start(S_sb[:], arS_out[:, :])
            nc.sync.dma_start(M_sb[:], arM_out[:, :])

            # batchnorm (over the B free dim), per 128-feature tile
            def bn(t_sb, gam, bet):
                mu = wpool.tile([PART, 1], f32, tag="mu")
                nc.vector.tensor_reduce(out=mu[:], in_=t_sb[:], axis=AX.X,
                                        op=ALU.add)
                nc.vector.tensor_scalar_mul(mu[:], mu[:], 1.0 / B)
                nc.vector.tensor_scalar(out=t_sb[:], in0=t_sb[:],
                                        scalar1=mu[:], scalar2=None,
                                        op0=ALU.subtract)
                sq = wpool.tile([PART, B], f32, tag="sq")
                nc.vector.tensor_tensor(out=sq[:], in0=t_sb[:], in1=t_sb[:],
                                        op=ALU.mult)
                var = wpool.tile([PART, 1], f32, tag="var")
                nc.vector.tensor_reduce(out=var[:], in_=sq[:], axis=AX.X,
                                        op=ALU.add)
                nc.vector.tensor_scalar(out=var[:], in0=var[:],
                                        scalar1=1.0 / B, scalar2=EPS,
                                        op0=ALU.mult, op1=ALU.add)
                nc.scalar.activation(var[:], var[:], ACTF.Sqrt)
                nc.vector.reciprocal(var[:], var[:])
                nc.vector.tensor_scalar(out=t_sb[:], in0=t_sb[:],
                                        scalar1=var[:], scalar2=gam[:],
                                        op0=ALU.mult, op1=ALU.mult)
                nc.vector.tensor_scalar(out=t_sb[:], in0=t_sb[:],
                                        scalar1=bet[:], scalar2=None,
                                        op0=ALU.add)

            bn(S_sb, gS_s, bS_s)
            bn(M_sb, gM_s, bM_s)

            # lin1 [B, H] = S^T @ l1WS + M^T @ l1WM
            pl1 = pspool.tile([B, H], f32, tag="mm")
            nc.tensor.matmul(out=pl1[:], lhsT=S_sb[:], rhs=l1WS_s[:],
                             start=True, stop=False)
            nc.tensor.matmul(out=pl1[:], lhsT=M_sb[:], rhs=l1WM_s[:],
                             start=False, stop=True)
            y = wpool.tile([B, H], f32, tag="y")
            nc.vector.tensor_tensor(out=y[:], in0=pl1[:], in1=l1b_s[:B, :],
                                    op=ALU.add)
            nc.scalar.activation(y[:], y[:], ACTF.Relu)
            yT_ps = pspool.tile([H, B], f32, tag="tp")
            nc.tensor.matmul(out=yT_ps[:], lhsT=y[:], rhs=ident[:B, :B],
                             start=True, stop=True)
            yT = wpool.tile([H, B], f32, tag="yTs")
            nc.scalar.copy(out=yT[:], in_=yT_ps[:])
            pl2 = pspool.tile([B, NCLS], f32, tag="mm")
            nc.tensor.matmul(out=pl2[:], lhsT=yT[:], rhs=l2W_s[:],
                             start=True, stop=True)
            z = wpool.tile([B, NCLS], f32, tag="z")
            nc.vector.tensor_tensor(out=z[:], in0=pl2[:], in1=l2b_s[:B, :],
                                    op=ALU.add)
            zmax = wpool.tile([B, 1], f32, tag="zmax")
            nc.vector.tensor_reduce(out=zmax[:], in_=z[:], axis=AX.X,
                                    op=ALU.max)
            nc.vector.tensor_scalar(out=z[:], in0=z[:], scalar1=zmax[:],
                                    scalar2=None, op0=ALU.subtract)
            nc.scalar.activation(z[:], z[:], ACTF.Exp)
            zsum = wpool.tile([B, 1], f32, tag="zsum")
            nc.vector.tensor_reduce(out=zsum[:], in_=z[:], axis=AX.X,
                                    op=ALU.add)
            nc.vector.reciprocal(zsum[:], zsum[:])
            nc.vector.tensor_scalar(out=z[:], in0=z[:], scalar1=zsum[:],
                                    scalar2=None, op0=ALU.mult)
            nc.sync.dma_start(out_ext[:, :], z[:])
            sc_head.__exit__(None, None, None)

    nc.compile()
    return nc


def kernel(**inputs):
    from concourse import bass_utils
    meta, in_maps = _prep(inputs)
    nc = _build(meta)
    res = bass_utils.run_bass_kernel_spmd(
        nc, in_maps, core_ids=list(range(NCORES)))
    return np.asarray(res.results[0]["out"])



# revision 14
# speedup vs baseline: 1.0875x; 1.0875x over previous
"""Bass/Trainium2 kernel for nn_KPlexPool (GCN blocks + cover pooling), 8 NeuronCores.

Strategy: shard nodes/clusters across 8 cores by slicing each graph's
(batch-sorted) range 8 ways, degree-sorted within slices for padded-CSR
efficiency.  Self-loops are folded into the CSR on the host so each GCN layer
is: per-shard matmul h=x@W (TensorE, bf16), scale rows by dis=rsqrt(deg) ->
AllGather bf16 hs -> one multi-column indirect-DMA gather per chunk (amortizes
the ~1us SWDGE fixed cost over up to 8K rows) -> DVE multiply by edge weight +
segmented reduce -> scale/bias/relu epilogue written straight into an
SBUF-resident activation buffer (so the next matmul needs no DRAM round trip).
Cover pooling (sum+max) and batch readouts reuse the same machinery.  All
gathered tables are bf16 (halves HBM traffic; tolerance is 2e-2).  Tiny
AllReduces combine the [16,256] readout; every core computes the final
BN+MLP+softmax head identically.  All index structure is host-prepared.
"""

import sys
import numpy as np
import ml_dtypes

sys.path.insert(0, "/opt/trn_rl_repo")

PART = 128
NCORES = 8
EPS = 1e-5
S_COLS = 64          # max slot-columns per gather chunk (128*64 = 8192 rows)
G_MAX = 12           # max tiles per gather chunk
STRIPE = 32          # tiles per staging stripe (matmul output -> DRAM DMA)

BF16 = ml_dtypes.bfloat16


# ----------------------------------------------------------------- host prep

def _shard_items(batch, sortkey, B):
    """Slice each graph's contiguous (sorted-batch) range into 8 balanced
    parts; within a slice order items by sortkey desc; pad each graph-slice to
    a uniform (across cores) multiple of 128 rows.

    Returns perm [NCORES][rows] (orig ids, -1 pad), pos [n] -> global row,
    rows, tile_graph [T_tot]."""
    n = batch.shape[0]
    counts = np.bincount(batch, minlength=B)
    starts = np.concatenate([[0], np.cumsum(counts)[:-1]])
    perm_cores = [[] for _ in range(NCORES)]
    tile_graph = []
    for g in range(B):
        cnt = int(counts[g])
        st = int(starts[g])
        base, rem = divmod(cnt, NCORES)
        sizes = [base + (1 if c < rem else 0) for c in range(NCORES)]
        Tg = max(1, -(-max(sizes) // PART))
        tile_graph += [g] * Tg
        off = st
        for c in range(NCORES):
            s = sizes[c]
            ids = np.arange(off, off + s)
            off += s
            order = np.argsort(-sortkey[ids], kind="stable")
            padded = np.full(Tg * PART, -1, dtype=np.int64)
            padded[:s] = ids[order]
            perm_cores[c].append(padded)
    perm = [np.concatenate(p) for p in perm_cores]
    rows = perm[0].shape[0]
    pos = np.full(n, -1, dtype=np.int64)
    for c in range(NCORES):
        real = perm[c] >= 0
        pos[perm[c][real]] = c * rows + np.nonzero(real)[0]
    return perm, pos, rows, np.asarray(tile_graph)


def _csr_build(dst_local_all, src_pos_all, w_all, owner_all, rows, pad_idx):
    """Build per-core padded CSR. dst_local_all/src_pos_all/w_all: per-edge
    (global) arrays; owner_all: owning core per edge. Returns k_t (uniform),
    col_off, and per-core (idx [128,S], ew [128,S]) arrays."""
    T = rows // PART
    k_core = np.zeros((NCORES, T), dtype=np.int64)
    per_core = []
    for c in range(NCORES):
        m = owner_all == c
        dl = dst_local_all[m]
        sp = src_pos_all[m]
        w = w_all[m] if w_all is not None else None
        cnt = np.bincount(dl, minlength=rows)
        k_core[c] = cnt.reshape(T, PART).max(axis=1)
        per_core.append((dl, sp, w, cnt))
    k_t = np.maximum(k_core.max(axis=0), 1)
    col_off = np.concatenate([[0], np.cumsum(k_t)]).astype(np.int64)
    S = int(col_off[-1])
    idxs, ews = [], []
    for c in range(NCORES):
        dl, sp, w, cnt = per_core[c]
        order = np.argsort(dl, kind="stable")
        dls, sps = dl[order], sp[order]
        first = np.concatenate([[0], np.cumsum(cnt)[:-1]])
        rank = np.arange(dls.shape[0]) - first[dls]
        tile = dls // PART
        p = dls % PART
        col = col_off[tile] + rank
        idx = np.full((PART, S), pad_idx, dtype=np.int32)
        ew = np.zeros((PART, S), dtype=np.float32)
        idx[p, col] = sps.astype(np.int32)
        if w is not None:
            ew[p, col] = w[order]
        idxs.append(idx)
        ews.append(ew)
    return k_t, col_off, S, idxs, ews


def _chunks(k_t, col_off):
    """Greedy grouping of consecutive tiles into gather chunks."""
    out = []
    t0 = 0
    T = len(k_t)
    while t0 < T:
        t1 = t0
        while (t1 < T and col_off[t1 + 1] - col_off[t0] <= S_COLS
               and t1 - t0 < G_MAX):
            t1 += 1
        assert t1 > t0, f"tile {t0} k={k_t[t0]} exceeds S_COLS={S_COLS}"
        out.append((t0, t1, int(col_off[t0]), int(col_off[t1])))
        t0 = t1
    return out


def _prep(inputs):
    f32 = np.float32
    x = np.asarray(inputs["x"], f32)
    ei = np.asarray(inputs["edge_index"], np.int64)
    wts = np.asarray(inputs["weights"], f32)
    batch = np.asarray(inputs["batch"], np.int64)
    cover_n = np.asarray(inputs["cover_n"], np.int64)
    cover_c = np.asarray(inputs["cover_c"], np.int64)
    ei2 = np.asarray(inputs["edge_index2"], np.int64)
    wts2 = np.asarray(inputs["weights2"], f32)
    batch2 = np.asarray(inputs["batch2"], np.int64)
    N = x.shape[0]
    C = batch2.shape[0]
    B = int(batch.max()) + 1 if batch.size else 1
    B = max(B, int(batch2.max()) + 1)

    indeg = np.bincount(ei[1], minlength=N)
    perm1, pos1, rows1, tg1 = _shard_items(batch, indeg, B)
    key2 = np.bincount(ei2[1], minlength=C)
    perm2, pos2, rows2, tg2 = _shard_items(batch2, key2, B)
    T1, T2 = rows1 // PART, rows2 // PART

    # a guaranteed pad (zero) row in the node arrangement
    zero_pos = None
    for c in range(NCORES):
        pads = np.nonzero(perm1[c] < 0)[0]
        if pads.size:
            zero_pos = c * rows1 + int(pads[0])
            break
    assert zero_pos is not None

    # big-graph CSR (dst = col = ei[1]); self-loops handled on-chip from SBUF
    dpos = pos1[ei[1]]
    owner = dpos // rows1
    dloc = dpos % rows1
    k1, coff1, S1, idx1, ew1 = _csr_build(dloc, pos1[ei[0]], wts, owner,
                                          rows1, 0)

    # cover CSR (dest = cluster), sources are node positions into x1_full
    cpos = pos2[cover_c]
    ownc = cpos // rows2
    clol = cpos % rows2
    kc, coffc, Sc, idxc, _ = _csr_build(clol, pos1[cover_n], None, ownc, rows2,
                                        zero_pos)

    # pooled-graph CSR
    dpos2 = pos2[ei2[1]]
    own2 = dpos2 // rows2
    dlo2 = dpos2 % rows2
    k2, coff2, S2, idx2, ew2 = _csr_build(dlo2, pos2[ei2[0]], wts2, own2,
                                          rows2, 0)

    # per-core dense transposed inputs (bf16) and masks
    xTs, m1s, m2s = [], [], []
    for c in range(NCORES):
        pc = perm1[c]
        xc = np.zeros((rows1, x.shape[1]), f32)
        xc[pc >= 0] = x[pc[pc >= 0]]
        xTs.append(np.ascontiguousarray(xc.T).astype(BF16))
        m1s.append(np.ascontiguousarray(
            (pc >= 0).astype(f32).reshape(T1, PART).T))
        p2 = perm2[c]
        m2s.append(np.ascontiguousarray(
            (p2 >= 0).astype(f32).reshape(T2, PART).T))

    meta = dict(B=B, T1=T1, T2=T2, rows1=rows1, rows2=rows2,
                k1=k1, coff1=coff1, S1=S1, ch1=_chunks(k1, coff1),
                kc=kc, coffc=coffc, Sc=Sc, chc=_chunks(kc, coffc),
                k2=k2, coff2=coff2, S2=S2, ch2=_chunks(k2, coff2),
                tg1=tg1, tg2=tg2, FIN=x.shape[1])

    # weights (bf16), replicated biases, permuted head params
    rep = lambda v: np.ascontiguousarray(
        np.broadcast_to(np.asarray(v, f32).reshape(1, -1), (PART, v.shape[-1])))
    g = np.asarray(inputs["bn_gamma"], f32)
    bb = np.asarray(inputs["bn_beta"], f32)
    l1w = np.asarray(inputs["lin1_W"], f32)
    H = np.asarray(inputs["W_in0"], f32).shape[1]
    selS = np.r_[0:H, 2 * H:3 * H]
    selM = np.r_[H:2 * H, 3 * H:4 * H]
    wbf = lambda n: np.asarray(inputs[n], f32).astype(BF16)
    shared = {
        "W_in0": wbf("W_in0"), "W_in1": wbf("W_in1"), "Wl_in": wbf("Wl_in"),
        "W_b0": wbf("W_b0"), "W_b1": wbf("W_b1"), "Wl_b": wbf("Wl_b"),
        "b_in0": rep(inputs["b_in0"]), "b_in1": rep(inputs["b_in1"]),
        "bl_in": rep(inputs["bl_in"]), "b_b0": rep(inputs["b_b0"]),
        "b_b1": rep(inputs["b_b1"]), "bl_b": rep(inputs["bl_b"]),
        "gammaS": np.ascontiguousarray(g[selS].reshape(PART, 1)),
        "gammaM": np.ascontiguousarray(g[selM].reshape(PART, 1)),
        "betaS": np.ascontiguousarray(bb[selS].reshape(PART, 1)),
        "betaM": np.ascontiguousarray(bb[selM].reshape(PART, 1)),
        "l1WS": np.ascontiguousarray(l1w[selS]),
        "l1WM": np.ascontiguousarray(l1w[selM]),
        "l1b": rep(inputs["lin1_b"]),
        "l2W": np.asarray(inputs["lin2_W"], f32),
        "l2b": rep(inputs["lin2_b"]),
    }
    in_maps = []
    for c in range(NCORES):
        m = dict(shared)
        m["x_cT"] = xTs[c]
        m["mask1"] = m1s[c]
        m["mask2"] = m2s[c]
        m["idx1"] = idx1[c]
        m["ew1"] = ew1[c].astype(BF16)
        m["idxc"] = idxc[c]
        m["idx2"] = idx2[c]
        m["ew2"] = ew2[c].astype(BF16)
        in_maps.append(m)
    return meta, in_maps


# ------------------------------------------------------------- device kernel

def _build(meta, NCLS=10, H=64):
    import concourse.bass as bass
    import concourse.bacc as bacc
    import concourse.mybir as mybir
    import concourse.tile as tile
    from concourse.masks import make_identity

    f32 = mybir.dt.float32
    bf16 = mybir.dt.bfloat16
    i32 = mybir.dt.int32
    ALU = mybir.AluOpType
    ACTF = mybir.ActivationFunctionType
    AX = mybir.AxisListType
    IOA = bass.IndirectOffsetOnAxis

    B = meta["B"]
    T1, T2 = meta["T1"], meta["T2"]
    rows1, rows2 = meta["rows1"], meta["rows2"]
    FIN = meta["FIN"]
    RG = [list(range(NCORES))]

    nc = bacc.Bacc("TRN2", target_bir_lowering=False, debug=False,
                   num_devices=NCORES, num_swdge_queues=4)

    # Round-robin indirect DMAs across the 4 SWDGE queues.
    _qctr = [0]

    def _q(inst):
        i = _qctr[0] % 4
        _qctr[0] += 1
        if i:
            inst.ins.queue = f"qPoolDynamic{i}"
        return inst

    # ---- I/O ----
    ein = lambda n, s, d=f32: nc.dram_tensor(n, s, d, kind="ExternalInput")
    x_cT = ein("x_cT", [FIN, rows1], bf16)
    mask1 = ein("mask1", [PART, T1])
    mask2 = ein("mask2", [PART, T2])
    idx1 = ein("idx1", [PART, meta["S1"]], i32)
    ew1 = ein("ew1", [PART, meta["S1"]], bf16)
    idxc = ein("idxc", [PART, meta["Sc"]], i32)
    idx2 = ein("idx2", [PART, meta["S2"]], i32)
    ew2 = ein("ew2", [PART, meta["S2"]], bf16)
    wshapes = {"W_in0": [FIN, H], "W_in1": [H, H], "Wl_in": [2 * H, H],
               "W_b0": [2 * H, H], "W_b1": [H, H], "Wl_b": [2 * H, H]}
    Ws = {n: nc.dram_tensor(n, s, bf16, kind="ExternalInput")
          for n, s in wshapes.items()}
    bs = {n: ein(n, [PART, H]) for n in
          ("b_in0", "b_in1", "bl_in", "b_b0", "b_b1", "bl_b")}
    gammaS = ein("gammaS", [PART, 1]); gammaM = ein("gammaM", [PART, 1])
    betaS = ein("betaS", [PART, 1]); betaM = ein("betaM", [PART, 1])
    l1WS = ein("l1WS", [PART, H]); l1WM = ein("l1WM", [PART, H])
    l1b = ein("l1b", [PART, H])
    l2W = ein("l2W", [H, NCLS]); l2b = ein("l2b", [PART, NCLS])
    out_ext = nc.dram_tensor("out", [B, NCLS], f32, kind="ExternalOutput")

    # ---- internal DRAM ----
    hs_c1 = nc.dram_tensor("hs_c1", [rows1, H], bf16)
    hs_full1 = nc.dram_tensor("hs_full1", [NCORES * rows1, H], bf16, addr_space="Shared")
    hs_c1b = nc.dram_tensor("hs_c1b", [rows1, H], bf16)
    hs_full1b = nc.dram_tensor("hs_full1b", [NCORES * rows1, H], bf16, addr_space="Shared")
    x1_c = nc.dram_tensor("x1_c", [rows1, H], bf16)
    x1_full = nc.dram_tensor("x1_full", [NCORES * rows1, H], bf16, addr_space="Shared")
    hs_c2 = nc.dram_tensor("hs_c2", [rows2, H], bf16)
    hs_full2 = nc.dram_tensor("hs_full2", [NCORES * rows2, H], bf16, addr_space="Shared")
    hs_c2b = nc.dram_tensor("hs_c2b", [rows2, H], bf16)
    hs_full2b = nc.dram_tensor("hs_full2b", [NCORES * rows2, H], bf16, addr_space="Shared")
    arS_in = nc.dram_tensor("arS_in", [PART, B], f32)
    arS_out = nc.dram_tensor("arS_out", [PART, B], f32, addr_space="Shared")
    arM_in = nc.dram_tensor("arM_in", [PART, B], f32)
    arM_out = nc.dram_tensor("arM_out", [PART, B], f32, addr_space="Shared")

    with tile.TileContext(nc) as tc:
        with (tc.tile_pool(name="const", bufs=1) as cpool,
              tc.tile_pool(name="res", bufs=1) as rpool,
              tc.tile_pool(name="gtp", bufs=3) as gtpool,
              tc.tile_pool(name="stg", bufs=2) as stgpool,
              tc.tile_pool(name="work", bufs=3) as wpool,
              tc.tile_pool(name="ps", bufs=3, space="PSUM") as pspool,
              tc.tile_pool(name="psacc", bufs=1, space="PSUM") as papool):

            ident = cpool.tile([PART, PART], f32, tag="ident")
            make_identity(nc, ident[:])
            identB = cpool.tile([PART, PART], bf16, tag="identB")
            make_identity(nc, identB[:])

            def load2d(dram, shape, dt=f32, tag=None):
                t = cpool.tile(list(shape), dt, tag=tag or dram.name)
                nc.sync.dma_start(t[:], dram[:, :])
                return t

            idx1_s = load2d(idx1, (PART, meta["S1"]), i32)
            ew1_s = load2d(ew1, (PART, meta["S1"]), bf16)
            idxc_s = load2d(idxc, (PART, meta["Sc"]), i32)
            idx2_s = load2d(idx2, (PART, meta["S2"]), i32)
            ew2_s = load2d(ew2, (PART, meta["S2"]), bf16)
            mask1_s = load2d(mask1, (PART, T1))
            mask2_s = load2d(mask2, (PART, T2))
            W_s = {n: load2d(Ws[n], Ws[n].shape, bf16) for n in Ws}
            b_s = {n: load2d(bs[n], (PART, H)) for n in bs}
            l1WS_s = load2d(l1WS, (PART, H)); l1WM_s = load2d(l1WM, (PART, H))
            l1b_s = load2d(l1b, (PART, H))
            l2W_s = load2d(l2W, (H, NCLS)); l2b_s = load2d(l2b, (PART, NCLS))
            gS_s = load2d(gammaS, (PART, 1)); gM_s = load2d(gammaM, (PART, 1))
            bS_s = load2d(betaS, (PART, 1)); bM_s = load2d(betaM, (PART, 1))

            dis1 = rpool.tile([PART, T1], f32, tag="dis1")
            dis2 = rpool.tile([PART, T2], f32, tag="dis2")
            gdNT = rpool.tile([PART, T1, 2 * H], bf16, tag="gdNT")
            gd2NT = rpool.tile([PART, T2, 2 * H], bf16, tag="gd2NT")
            xpNT = rpool.tile([PART, T2, 2 * H], bf16, tag="xpNT")
            hs1_sb = rpool.tile([PART, T1, H], bf16, tag="hs1_sb")
            hs2_sb = rpool.tile([PART, T2, H], bf16, tag="hs2_sb")
            rm1 = rpool.tile([PART, B, H], f32, tag="rm1")
            rm2 = rpool.tile([PART, B, H], f32, tag="rm2")
            oneh = rpool.tile([PART, B, B], f32, tag="oneh")
            nc.vector.memset(rm1[:], 0.0)
            nc.vector.memset(rm2[:], 0.0)
            nc.vector.memset(oneh[:], 0.0)
            for g in range(B):
                nc.vector.memset(oneh[:, g, g:g + 1], 1.0)

            def bc_mid(ap2d, G):
                """[128, F] AP -> [128, (0-step G), F] broadcast view."""
                a = ap2d.ap
                return bass.AP(ap2d.tensor, ap2d.offset,
                               [a[0], [0, G], a[-1]])

            ps_sum1 = papool.tile([B, H], f32, tag="sum1")
            ps_sum2 = papool.tile([B, H], f32, tag="sum2")

            # degree -> dis (= rsqrt(deg) * mask); +1 for the self-loop
            def make_dis(ew_s, k, coff, T, mask_s, dis_t):
                deg = wpool.tile([PART, max(T1, T2)], f32, tag="deg")
                for t in range(T):
                    kk = int(k[t])
                    nc.vector.tensor_reduce(
                        out=deg[:, t:t + 1],
                        in_=ew_s[:, int(coff[t]):int(coff[t]) + kk],
                        axis=AX.X, op=ALU.add)
                nc.vector.tensor_scalar_add(deg[:, :T], deg[:, :T], 1.0)
                nc.vector.reciprocal(deg[:, :T], deg[:, :T])
                nc.scalar.activation(deg[:, :T], deg[:, :T], ACTF.Sqrt)
                nc.vector.tensor_tensor(out=dis_t[:], in0=deg[:, :T],
                                        in1=mask_s[:], op=ALU.mult)

            make_dis(ew1_s, meta["k1"], meta["coff1"], T1, mask1_s, dis1)
            make_dis(ew2_s, meta["k2"], meta["coff2"], T2, mask2_s, dis2)

            def stripes(T):
                return [(s, min(s + STRIPE, T)) for s in range(0, T, STRIPE)]

            # h = act @ W scaled by dis -> persistent SBUF hs (bf16, also the
            # self-loop source for mp) -> striped DMA to DRAM hs for AllGather.
            # lhsT_fn(t) returns a bf16 [k, 128] SBUF AP for tile t (may issue
            # transpose+copy instructions).
            def mm_phase(lhsT_fn, Tn, W, dis_t, hs_sb, hs_dram):
                hsd = hs_dram.ap().rearrange("(t p) f -> p t f", p=PART)
                for (s0, s1) in stripes(Tn):
                    for t in range(s0, s1):
                        lhsT = lhsT_fn(t)
                        mm = pspool.tile([PART, H], f32, tag="mm")
                        nc.tensor.matmul(out=mm[:], lhsT=lhsT, rhs=W[:],
                                         start=True, stop=True)
                        nc.vector.tensor_scalar(
                            out=hs_sb[:, t, :], in0=mm[:],
                            scalar1=dis_t[:, t:t + 1], scalar2=None,
                            op0=ALU.mult)
                    nc.sync.dma_start(hsd[:, s0:s1, :], hs_sb[:, s0:s1, :])

            def lhsT_transpose(src_nt, half):
                """tile t -> transpose src_nt[:, t, half] -> bf16 [k, 128]."""
                def fn(t):
                    src = (src_nt[:, t, :] if half is None
                           else src_nt[:, t, half * H:(half + 1) * H])
                    kdim = src.shape[-1]
                    tp = pspool.tile([PART, PART], bf16, tag="tp")
                    nc.tensor.transpose(tp[:kdim, :], src, identB[:])
                    tsb = wpool.tile([PART, PART], bf16, tag="tsb")
                    nc.scalar.copy(out=tsb[:kdim, :], in_=tp[:kdim, :])
                    return tsb[:kdim, :]
                return fn

            def allgather(src, dst):
                nc.gpsimd.collective_compute(
                    "AllGather", ALU.bypass, ins=[src.ap().opt()],
                    outs=[dst.ap().opt()], replica_groups=RG)

            # gather + ew-mult + per-tile reduce + self-add + epilogue ->
            # relu-cast into dst_nt[:, t, half*H:(half+1)*H] (SBUF-resident)
            def mp_phase(hs_full, idx_s, ew_s, k, chunks, coff, dis_t,
                         hs_sb, bias, dst_nt, half):
                for (t0, t1, c0, c1) in chunks:
                    Wc = c1 - c0
                    G = t1 - t0
                    gt = gtpool.tile([PART, S_COLS, H], bf16, tag="gt")
                    gv = gt[:, :Wc, :]
                    for cc in range(Wc):
                        _q(nc.gpsimd.indirect_dma_start(
                            out=gt[:, cc, :], out_offset=None,
                            in_=hs_full[:, :],
                            in_offset=IOA(ap=idx_s[:, c0 + cc:c0 + cc + 1],
                                          axis=0)))
                    nc.vector.tensor_tensor(
                        out=gv, in0=gv,
                        in1=ew_s[:, c0:c1].to_broadcast([PART, Wc, H]),
                        op=ALU.mult)
                    zv = wpool.tile([PART, G_MAX, H], f32, tag="zv")
                    for j, t in enumerate(range(t0, t1)):
                        ca = int(coff[t]) - c0
                        kk = int(k[t])
                        nc.vector.tensor_reduce(
                            out=zv[:, j, :],
                            in_=gt[:, ca:ca + kk, :].rearrange(
                                "p k f -> p f k"),
                            axis=AX.X, op=ALU.add)
                    nc.vector.tensor_tensor(out=zv[:, :G, :], in0=zv[:, :G, :],
                                            in1=hs_sb[:, t0:t1, :],
                                            op=ALU.add)
                    nc.vector.tensor_tensor(
                        out=zv[:, :G, :], in0=zv[:, :G, :],
                        in1=dis_t[:, t0:t1].to_broadcast([PART, G, H]),
                        op=ALU.mult)
                    nc.vector.tensor_tensor(out=zv[:, :G, :], in0=zv[:, :G, :],
                                            in1=bc_mid(bias[:], G), op=ALU.add)
                    for j, t in enumerate(range(t0, t1)):
                        nc.scalar.activation(
                            dst_nt[:, t, half * H:(half + 1) * H],
                            zv[:, j, :], ACTF.Relu)

            # JK cat + linear + relu*mask + readouts (+ optional bf16 store)
            def jk_phase(src_nt, Tn, Wl, bias, mask_s, tg, ps_sum, rm,
                         x_dram):
                lfn = lhsT_transpose(src_nt, None)
                xd = (x_dram.ap().rearrange("(t p) f -> p t f", p=PART)
                      if x_dram is not None else None)
                for (s0, s1) in stripes(Tn):
                    stg = (stgpool.tile([PART, STRIPE, H], bf16, tag="hstg",
                                        name="stg")
                           if x_dram is not None else None)
                    for t in range(s0, s1):
                        lhsT = lfn(t)
                        mm = pspool.tile([PART, H], f32, tag="mm")
                        nc.tensor.matmul(out=mm[:], lhsT=lhsT, rhs=Wl[:],
                                         start=True, stop=True)
                        xt = wpool.tile([PART, H], f32, tag="xt")
                        nc.vector.tensor_tensor(out=xt[:], in0=mm[:],
                                                in1=bias[:], op=ALU.add)
                        nc.scalar.activation(xt[:], xt[:], ACTF.Relu,
                                             scale=mask_s[:, t:t + 1])
                        g = int(tg[t])
                        nc.tensor.matmul(out=ps_sum[:], lhsT=oneh[:, g, :],
                                         rhs=xt[:], start=(t == 0),
                                         stop=(t == Tn - 1),
                                         skip_group_check=True)
                        nc.vector.tensor_tensor(out=rm[:, g, :],
                                                in0=rm[:, g, :],
                                                in1=xt[:], op=ALU.max)
                        if stg is not None:
                            nc.scalar.copy(out=stg[:, t - s0, :], in_=xt[:])
                    if stg is not None:
                        nc.sync.dma_start(xd[:, s0:s1, :], stg[:, :s1 - s0, :])

            xTv = x_cT.ap()
            _xcache = {}

            def lhsT_x(t):
                s0 = (t // STRIPE) * STRIPE
                if s0 not in _xcache:
                    xstg = stgpool.tile([FIN, STRIPE * PART], bf16, tag="xstg")
                    s1 = min(s0 + STRIPE, T1)
                    nc.sync.dma_start(xstg[:, :(s1 - s0) * PART],
                                      xTv[:, s0 * PART:s1 * PART])
                    _xcache[s0] = xstg
                return _xcache[s0][:, (t - s0) * PART:(t - s0 + 1) * PART]

            # ---------- big block ----------
            with nc.named_scope("mm1a"):
                mm_phase(lhsT_x, T1, W_s["W_in0"], dis1, hs1_sb, hs_c1)
            with nc.named_scope("ag1a"):
                allgather(hs_c1, hs_full1)
            with nc.named_scope("mp1a"):
                mp_phase(hs_full1, idx1_s, ew1_s, meta["k1"], meta["ch1"],
                         meta["coff1"], dis1, hs1_sb, b_s["b_in0"], gdNT, 0)
            with nc.named_scope("mm1b"):
                mm_phase(lhsT_transpose(gdNT, 0), T1, W_s["W_in1"], dis1,
                         hs1_sb, hs_c1b)
            with nc.named_scope("ag1b"):
                allgather(hs_c1b, hs_full1b)
            with nc.named_scope("mp1b"):
                mp_phase(hs_full1b, idx1_s, ew1_s, meta["k1"], meta["ch1"],
                         meta["coff1"], dis1, hs1_sb, b_s["b_in1"], gdNT, 1)
            with nc.named_scope("jk1"):
                jk_phase(gdNT, T1, W_s["Wl_in"], b_s["bl_in"], mask1_s,
                         meta["tg1"], ps_sum1, rm1, x1_c)
            with nc.named_scope("agx1"):
                allgather(x1_c, x1_full)

            # ---------- cover pooling (sum+max into xpNT) ----------
            with nc.named_scope("cover"):
                for (t0, t1, c0, c1) in meta["chc"]:
                    Wc = c1 - c0
                    gt = gtpool.tile([PART, S_COLS, H], bf16, tag="gt")
                    for cc in range(Wc):
                        _q(nc.gpsimd.indirect_dma_start(
                            out=gt[:, cc, :], out_offset=None,
                            in_=x1_full[:, :],
                            in_offset=IOA(ap=idxc_s[:, c0 + cc:c0 + cc + 1],
                                          axis=0)))
                    zv = wpool.tile([PART, G_MAX, H], f32, tag="zv")
                    for j, t in enumerate(range(t0, t1)):
                        ca = int(meta["coffc"][t]) - c0
                        kk = int(meta["kc"][t])
                        view = gt[:, ca:ca + kk, :].rearrange("p k f -> p f k")
                        nc.vector.tensor_reduce(out=zv[:, j, :], in_=view,
                                                axis=AX.X, op=ALU.add)
                        nc.scalar.copy(out=xpNT[:, t, 0:H], in_=zv[:, j, :])
                        nc.vector.tensor_reduce(out=xpNT[:, t, H:2 * H],
                                                in_=view, axis=AX.X,
                                                op=ALU.max)

            # ---------- pooled block ----------
            with nc.named_scope("mm2a"):
                mm_phase(lhsT_transpose(xpNT, None), T2, W_s["W_b0"], dis2,
                         hs2_sb, hs_c2)
            with nc.named_scope("ag2a"):
                allgather(hs_c2, hs_full2)
            with nc.named_scope("mp2a"):
                mp_phase(hs_full2, idx2_s, ew2_s, meta["k2"], meta["ch2"],
                         meta["coff2"], dis2, hs2_sb, b_s["b_b0"], gd2NT, 0)
            with nc.named_scope("mm2b"):
                mm_phase(lhsT_transpose(gd2NT, 0), T2, W_s["W_b1"], dis2,
                         hs2_sb, hs_c2b)
            with nc.named_scope("ag2b"):
                allgather(hs_c2b, hs_full2b)
            with nc.named_scope("mp2b"):
                mp_phase(hs_full2b, idx2_s, ew2_s, meta["k2"], meta["ch2"],
                         meta["coff2"], dis2, hs2_sb, b_s["b_b1"], gd2NT, 1)
            with nc.named_scope("jk2"):
                jk_phase(gd2NT, T2, W_s["Wl_b"], b_s["bl_b"], mask2_s,
                         meta["tg2"], ps_sum2, rm2, None)

            # ---------- readout combine + head ----------
            sc_head = nc.named_scope("head"); sc_head.__enter__()
            sum1_sb = wpool.tile([B, H], f32, tag="s1sb")
            nc.scalar.copy(out=sum1_sb[:], in_=ps_sum1[:])
            sum2_sb = wpool.tile([B, H], f32, tag="s2sb")
            nc.scalar.copy(out=sum2_sb[:], in_=ps_sum2[:])
            sT = pspool.tile([H, B], f32, tag="tp")
            nc.tensor.matmul(out=sT[:], lhsT=sum1_sb[:], rhs=ident[:B, :B],
                             start=True, stop=True)
            sT1 = wpool.tile([H, B], f32, tag="sT1")
            nc.scalar.copy(out=sT1[:], in_=sT[:])
            sT_2 = pspool.tile([H, B], f32, tag="tp")
            nc.tensor.matmul(out=sT_2[:], lhsT=sum2_sb[:], rhs=ident[:B, :B],
                             start=True, stop=True)
            sT2 = wpool.tile([H, B], f32, tag="sT2")
            nc.scalar.copy(out=sT2[:], in_=sT_2[:])
            nc.sync.dma_start(arS_in[0:H, :], sT1[:])
            nc.sync.dma_start(arS_in[H:2 * H, :], sT2[:])

            # maxes: rm[g] [128, H] -> transpose -> reduce-max -> [H, 1]
            mT1 = wpool.tile([H, B], f32, tag="mT1")
            mT2 = wpool.tile([H, B], f32, tag="mT2")
            for g in range(B):
                for rm, mt in ((rm1, mT1), (rm2, mT2)):
                    tpm = pspool.tile([H, PART], f32, tag="tp")
                    nc.tensor.transpose(tpm[:], rm[:, g, :], ident[:])
                    msb = wpool.tile([H, PART], f32, tag="msb")
                    nc.scalar.copy(out=msb[:], in_=tpm[:])
                    nc.vector.tensor_reduce(out=mt[:, g:g + 1], in_=msb[:],
                                            axis=AX.X, op=ALU.max)
            nc.sync.dma_start(arM_in[0:H, :], mT1[:])
            nc.sync.dma_start(arM_in[H:2 * H, :], mT2[:])

            nc.gpsimd.collective_compute(
                "AllReduce", ALU.add, ins=[arS_in.ap().opt()],
                outs=[arS_out.ap().opt()], replica_groups=RG)
            nc.gpsimd.collective_compute(
                "AllReduce", ALU.max, ins=[arM_in.ap().opt()],
                outs=[arM_out.ap().opt()], replica_groups=RG)

            S_sb = wpool.tile([PART, B], f32, tag="Ssb")
            M_sb = wpool.tile([PART, B], f32, tag="Msb")
            nc.sync.dma_start(S_sb[:], arS_out[:, :])
            nc.sync.dma_start(M_sb[:], arM_out[:, :])

            # batchnorm (over the B free dim), per 128-feature tile
            def bn(t_sb, gam, bet):
                mu = wpool.tile([PART, 1], f32, tag="mu")
                nc.vector.tensor_reduce(out=mu[:], in_=t_sb[:], axis=AX.X,
                                        op=ALU.add)
                nc.vector.tensor_scalar_mul(mu[:], mu[:], 1.0 / B)
                nc.vector.tensor_scalar(out=t_sb[:], in0=t_sb[:],
                                        scalar1=mu[:], scalar2=None,
                                        op0=ALU.subtract)
                sq = wpool.tile([PART, B], f32, tag="sq")
                nc.vector.tensor_tensor(out=sq[:], in0=t_sb[:], in1=t_sb[:],
                                        op=ALU.mult)
                var = wpool.tile([PART, 1], f32, tag="var")
                nc.vector.tensor_reduce(out=var[:], in_=sq[:], axis=AX.X,
                                        op=ALU.add)
                nc.vector.tensor_scalar(out=var[:], in0=var[:],
                                        scalar1=1.0 / B, scalar2=EPS,
                                        op0=ALU.mult, op1=ALU.add)
                nc.scalar.activation(var[:], var[:], ACTF.Sqrt)
                nc.vector.reciprocal(var[:], var[:])
                nc.vector.tensor_scalar(out=t_sb[:], in0=t_sb[:],
                                        scalar1=var[:], scalar2=gam[:],
                                        op0=ALU.mult, op1=ALU.mult)
                nc.vector.tensor_scalar(out=t_sb[:], in0=t_sb[:],
                                        scalar1=bet[:], scalar2=None,
                                        op0=ALU.add)

            bn(S_sb, gS_s, bS_s)
            bn(M_sb, gM_s, bM_s)

            # lin1 [B, H] = S^T @ l1WS + M^T @ l1WM
            pl1 = pspool.tile([B, H], f32, tag="mm")
            nc.tensor.matmul(out=pl1[:], lhsT=S_sb[:], rhs=l1WS_s[:],
                             start=True, stop=False)
            nc.tensor.matmul(out=pl1[:], lhsT=M_sb[:], rhs=l1WM_s[:],
                             start=False, stop=True)
            y = wpool.tile([B, H], f32, tag="y")
            nc.vector.tensor_tensor(out=y[:], in0=pl1[:], in1=l1b_s[:B, :],
                                    op=ALU.add)
            nc.scalar.activation(y[:], y[:], ACTF.Relu)
            yT_ps = pspool.tile([H, B], f32, tag="tp")
            nc.tensor.matmul(out=yT_ps[:], lhsT=y[:], rhs=ident[:B, :B],
                             start=True, stop=True)
            yT = wpool.tile([H, B], f32, tag="yTs")
            nc.scalar.copy(out=yT[:], in_=yT_ps[:])
            pl2 = pspool.tile([B, NCLS], f32, tag="mm")
            nc.tensor.matmul(out=pl2[:], lhsT=yT[:], rhs=l2W_s[:],
                             start=True, stop=True)
            z = wpool.tile([B, NCLS], f32, tag="z")
            nc.vector.tensor_tensor(out=z[:], in0=pl2[:], in1=l2b_s[:B, :],
                                    op=ALU.add)
            zmax = wpool.tile([B, 1], f32, tag="zmax")
            nc.vector.tensor_reduce(out=zmax[:], in_=z[:], axis=AX.X,
                                    op=ALU.max)
            nc.vector.tensor_scalar(out=z[:], in0=z[:], scalar1=zmax[:],
                                    scalar2=None, op0=ALU.subtract)
            nc.scalar.activation(z[:], z[:], ACTF.Exp)
            zsum = wpool.tile([B, 1], f32, tag="zsum")
            nc.vector.tensor_reduce(out=zsum[:], in_=z[:], axis=AX.X,
                                    op=ALU.add)
            nc.vector.reciprocal(zsum[:], zsum[:])
            nc.vector.tensor_scalar(out=z[:], in0=z[:], scalar1=zsum[:],
                                    scalar2=None, op0=ALU.mult)
            nc.sync.dma_start(out_ext[:, :], z[:])
            sc_head.__exit__(None, None, None)

    nc.compile()
    return nc


def kernel(**inputs):
    from concourse import bass_utils
    meta, in_maps = _prep(inputs)
    nc = _build(meta)
    res = bass_utils.run_bass_kernel_spmd(
        nc, in_maps, core_ids=list(range(NCORES)))
    return np.asarray(res.results[0]["out"])
